# revision 33
# baseline (speedup 1.0000x reference)
"""Sparse L1-distance attention (nn_L1AttnSparse) on 8 Trainium2 NeuronCores.

v2: bf16 tables + engine-balanced score pipeline + PE slot-sum.

Sharding: dst tokens split across 8 cores (256 each); per chunk of 128 dst:
  - gather k rows bf16 in dst-layout [p=dst, s, h*w] (one 4096-idx SWDGE gather)
  - d = k - q (TT, slots split DVE/Pool), |d| on Act, tree-sum over w via
    TT-adds (bf16 levels then f32 tail) -> scores L f32 [p, s, h]
  - E = exp(-L/8) bf16 on Act; den = sum_s E (DVE); rden = 1/den
  - E rearranged to edge layout [p=(s,d'), g, h] via a DRAM roundtrip
  - gather v rows bf16 in edge-layout [p=(s,d'), g, w*h] (v table w-major)
  - wv = vg * E_edge (TT); slot-sum on PE: psum[4g:4g+4,:] = onehot.T @ wv[:,g,:]
  - out = psum * rden -> bf16 [p, w, h]; host casts/transposes back
"""

import sys

sys.path.insert(0, "/opt/trn_rl_repo")

import ml_dtypes
import numpy as np

import concourse.bass as bass
import concourse.tile as tile
from concourse import bacc, mybir
from concourse.bass_utils import run_bass_kernel_spmd

BF16 = ml_dtypes.bfloat16

BS = 2
N_TOK = 2048
NH = 8
W = 64
S = 32  # dst_mxlen
HW = NH * W  # 512
N_CORES = 8
DT = N_TOK // N_CORES  # dst tokens per core = 256
CH = DT // 128  # chunks of 128 dst per batch = 2
G = 128 // 4  # dst groups of 4 per chunk = 32
NIDX = 128 * S  # 4096 gather indices per chunk
NG = 4  # gathers per table per chunk (NIDX/NG indices each)
G_POOL = 0  # per-slice v-groups whose weight-mult runs on Pool
SCALE = -1.0 / np.sqrt(W)  # -0.125


def _wrap_idx(flat):
    """int16 index list -> [128, n/16] tile layout: idx i at [i%16, i//16],
    replicated down the 8 groups of 16 partitions."""
    n = flat.shape[0]
    w16 = np.zeros((16, n // 16), dtype=np.int16)
    w16[np.arange(n) % 16, np.arange(n) // 16] = flat
    return np.tile(w16, (8, 1))


def _score_tree(veng, kgv, Fv, Lv):
    """Sum over w (64) of |d| for kgv [128, ss, NH, W] bf16 (in-place tree),
    f32 tail via Fv [128, ss, NH, 4], result into Lv [128, ss, NH, 1] f32."""
    add = mybir.AluOpType.add

    def tt(o, a, b):
        veng.tensor_tensor(out=o, in0=a, in1=b, op=add)

    for half in (32, 16, 8):
        tt(kgv[:, :, :, 0:half], kgv[:, :, :, 0:half], kgv[:, :, :, half : 2 * half])
    tt(Fv, kgv[:, :, :, 0:4], kgv[:, :, :, 4:8])
    tt(Fv[:, :, :, 0:2], Fv[:, :, :, 0:2], Fv[:, :, :, 2:4])
    tt(Lv, Fv[:, :, :, 0:1], Fv[:, :, :, 1:2])


def build_kernel():
    nc = bacc.Bacc(
        "TRN2", target_bir_lowering=False, debug=False, num_devices=N_CORES,
        dynamic_dma_scratch_size=16384 * 4,
    )
    f32 = mybir.dt.float32
    bf16 = mybir.dt.bfloat16
    i16 = mybir.dt.int16
    Alu = mybir.AluOpType

    kf = nc.dram_tensor("kf", [BS * N_TOK, HW], bf16, kind="ExternalInput").ap()
    vf = nc.dram_tensor("vf", [BS * N_TOK, HW], bf16, kind="ExternalInput").ap()
    qc = nc.dram_tensor("qc", [BS, CH, 128, HW], bf16, kind="ExternalInput").ap()
    ik = nc.dram_tensor("ik", [BS, CH, 128, NIDX // 16], i16, kind="ExternalInput").ap()
    iv = nc.dram_tensor("iv", [BS, CH, 128, NIDX // 16], i16, kind="ExternalInput").ap()
    oh = nc.dram_tensor("oh", [128, 8 * 32], bf16, kind="ExternalInput").ap()
    oc = nc.dram_tensor("oc", [BS, CH, 128, HW], bf16, kind="ExternalOutput").ap()
    # DRAM scratch for the dst->edge layout rearrange of E
    sc = nc.dram_tensor("sc", [BS, CH, 128, S * NH], bf16, kind="Internal").ap()

    with tile.TileContext(nc) as tc:
        with (
            tc.tile_pool(name="kp", bufs=2) as kp,
            tc.tile_pool(name="vp", bufs=2) as vp,
            tc.tile_pool(name="sm", bufs=2) as sm,
            tc.tile_pool(name="cst", bufs=1) as cst,
            tc.tile_pool(name="pp", bufs=2, space="PSUM") as pp,
        ):
            oh_t = cst.tile([128, 8 * 32], bf16, tag="oh")
            nc.sync.dma_start(out=oh_t[:], in_=oh)
            nsl = S // NG  # slots per gather
            ngl = G // NG  # groups per gather
            chunks = [(b, c) for b in range(BS) for c in range(CH)]
            NC_ = len(chunks)
            st = [dict() for _ in chunks]  # per-chunk tiles

            def _isl(gi):
                return slice(gi * (NIDX // NG // 16), (gi + 1) * (NIDX // NG // 16))

            def emit_load(ci):
                b, c = chunks[ci]
                t = st[ci]
                t["q"] = sm.tile([128, HW], bf16, tag="q", name="q_t")
                nc.sync.dma_start(out=t["q"][:], in_=qc[b, c])
                t["ik"] = sm.tile([128, NIDX // 16], i16, tag="ik", name="ikt")
                nc.sync.dma_start(out=t["ik"][:], in_=ik[b, c])
                t["iv"] = sm.tile([128, NIDX // 16], i16, tag="iv", name="ivt")
                nc.sync.dma_start(out=t["iv"][:], in_=iv[b, c])
                t["kg"] = kp.tile([128, S, HW], bf16, tag="kg", name="kg")
                t["vg"] = vp.tile([128, G, HW], bf16, tag="vg", name="vg")
                for gi in range(NG):
                    nc.gpsimd.dma_gather(
                        t["kg"][:, gi * nsl : (gi + 1) * nsl],
                        kf, t["ik"][:, _isl(gi)], NIDX // NG, NIDX // NG, HW,
                        queue_num=0,
                    )
                for gi in range(NG):
                    nc.gpsimd.dma_gather(
                        t["vg"][:, gi * ngl : (gi + 1) * ngl],
                        vf, t["iv"][:, _isl(gi)], NIDX // NG, NIDX // NG, HW,
                        queue_num=0,
                    )

            def emit_score(ci):
                b, c = chunks[ci]
                t = st[ci]
                kg, q_t = t["kg"], t["q"]
                kgv = kg[:].rearrange("p s (h w) -> p s h w", w=W)
                F = sm.tile([128, S, NH, 4], f32, tag="F")
                L = sm.tile([128, S, NH, 1], f32, tag="L")
                E = sm.tile([128, S * NH], bf16, tag="E")
                for gi in range(NG):
                    s0, s1 = gi * nsl, (gi + 1) * nsl
                    nc.vector.tensor_tensor(
                        out=kg[:, s0:s1],
                        in0=kg[:, s0:s1],
                        in1=q_t[:, None, :].to_broadcast([128, nsl, HW]),
                        op=Alu.subtract,
                    )
                    # |d| on Act (runs while DVE subs/trees other slices)
                    nc.scalar.activation(
                        out=kg[:, s0:s1].rearrange("p s hw -> p (s hw)"),
                        in_=kg[:, s0:s1].rearrange("p s hw -> p (s hw)"),
                        func=mybir.ActivationFunctionType.Abs,
                    )
                    _score_tree(nc.vector, kgv[:, s0:s1], F[:, s0:s1], L[:, s0:s1])
                # exp + DRAM write per slice, emitted after the abs ops so
                # Act's in-order stream doesn't serialize the slice pipeline
                for gi in range(NG):
                    s0, s1 = gi * nsl, (gi + 1) * nsl
                    nc.scalar.activation(
                        out=E[:, s0 * NH : s1 * NH],
                        in_=L[:, s0:s1].rearrange("p s h one -> p (s h one)"),
                        func=mybir.ActivationFunctionType.Exp,
                        scale=float(SCALE),
                    )
                    # on Act's queue so SP.SEQ stays free for input loads
                    nc.scalar.dma_start(
                        out=sc[b, c, :, s0 * NH : s1 * NH],
                        in_=E[:, s0 * NH : s1 * NH],
                    )
                den = sm.tile([128, NH], f32, tag="den")
                nc.vector.tensor_reduce(
                    out=den[:],
                    in_=E[:].rearrange("p (s h) -> p h s", h=NH),
                    axis=mybir.AxisListType.X,
                    op=Alu.add,
                )
                t["rden"] = sm.tile([128, NH], f32, tag="rden", name="rden")
                nc.vector.reciprocal(t["rden"][:], den[:])
                # edge layout p2 = d'*32 + s: each dp fills a contiguous
                # 32-partition block of Ee
                t["Ee"] = sm.tile([128, G, NH], bf16, tag="Ee", name="Ee")
                sc_r = sc[b, c].rearrange("(g dp) (s h) -> dp s g h", dp=4, h=NH)
                for dp in range(4):
                    nc.scalar.dma_start(
                        out=t["Ee"][32 * dp : 32 * dp + 32], in_=sc_r[dp]
                    )

            def emit_vphase(ci):
                b, c = chunks[ci]
                t = st[ci]
                vg, Ee, rden = t["vg"], t["Ee"], t["rden"]
                vgv = vg[:].rearrange("p g (w h) -> p g w h", h=NH)
                psA = pp.tile([64, HW], f32, tag="psA")
                psB = pp.tile([64, HW], f32, tag="psB")
                # all weight-mults first so the PE matmul stream is never
                # data-starved (starved matmuls reset the PE p-state ramp)
                for gi in range(NG):
                    g0 = gi * ngl
                    gp = g0 + ngl - G_POOL  # last G_POOL groups -> Pool
                    for eng, ga, gb in (
                        (nc.vector, g0, gp),
                        (nc.gpsimd, gp, g0 + ngl),
                    ):
                        if gb > ga:
                            eng.tensor_tensor(
                                out=vgv[:, ga:gb],
                                in0=vgv[:, ga:gb],
                                in1=Ee[:, ga:gb, None, :].to_broadcast(
                                    [128, gb - ga, W, NH]
                                ),
                                op=Alu.mult,
                            )
                for gi in range(NG):
                    g0 = gi * ngl
                    ps = psA if gi < 2 else psB
                    off = 32 * (gi % 2)
                    for j in range(ngl):
                        nc.tensor.matmul(
                            out=ps[off : off + 32, :],
                            lhsT=oh_t[:, 32 * j : 32 * j + 32],
                            rhs=vg[:, g0 + j, :],
                            start=(j == 0),
                            stop=(j == ngl - 1),
                        )
                out_t = sm.tile([128, HW], bf16, tag="out")
                for half, ps in ((0, psA), (1, psB)):
                    nc.vector.tensor_tensor(
                        out=out_t[64 * half : 64 * half + 64].rearrange(
                            "p (w h) -> p w h", h=NH
                        ),
                        in0=ps[:].rearrange("p (w h) -> p w h", h=NH),
                        in1=rden[64 * half : 64 * half + 64, None, :].to_broadcast(
                            [64, W, NH]
                        ),
                        op=Alu.mult,
                    )
                # output store on Act's queue so it never blocks SP's loads
                nc.scalar.dma_start(out=oc[b, c], in_=out_t[:])

            # skewed emission: score(i+1) lands before vphase(i) in every
            # engine's in-order stream, hiding the E-roundtrip latency
            emit_load(0)
            emit_score(0)
            emit_load(1)
            emit_score(1)
            for ci in range(2, NC_):
                emit_vphase(ci - 2)
                emit_load(ci)
                emit_score(ci)
            emit_vphase(NC_ - 2)
            emit_vphase(NC_ - 1)
    nc.compile()
    return nc


_NC_CACHE = None


def kernel(v, q, k, coo, dst_mxlen):
    global _NC_CACHE
    assert int(dst_mxlen) == S
    v = np.asarray(v, dtype=np.float32)
    q = np.asarray(q, dtype=np.float32)
    k = np.asarray(k, dtype=np.float32)
    coo = np.asarray(coo)

    # src table: srct[t, s] = src index of edge (dst=t, slot=s)
    srct = np.zeros((N_TOK, S), dtype=np.int64)
    srct[coo[:, 0], coo[:, 2]] = coo[:, 1]

    kf = np.ascontiguousarray(k.reshape(BS * N_TOK, HW)).astype(BF16)
    # v table rows stored w-major ([w, h] per row)
    vf = np.ascontiguousarray(
        v.transpose(0, 1, 3, 2).reshape(BS * N_TOK, HW)
    ).astype(BF16)
    # one-hot lhsT blocks for edge partitions p2 = dp*32 + s:
    # ohm[p2, go, go*4 + dp] = 1
    ohm = np.zeros((128, 8, 32), dtype=BF16)
    p2 = np.arange(128)
    for go in range(8):
        ohm[p2, go, go * 4 + p2 // 32] = 1.0
    ohm = ohm.reshape(128, 256)

    if _NC_CACHE is None:
        _NC_CACHE = build_kernel()
    nc = _NC_CACHE

    in_maps = []
    for core in range(N_CORES):
        lo = core * DT
        qcc = q[:, lo : lo + DT].reshape(BS, CH, 128, HW).astype(BF16)
        ikm = np.zeros((BS, CH, 128, NIDX // 16), dtype=np.int16)
        ivm = np.zeros((BS, CH, 128, NIDX // 16), dtype=np.int16)
        for b in range(BS):
            for c in range(CH):
                sl = srct[lo + c * 128 : lo + (c + 1) * 128]  # [128 dst, 32 s]
                # k-gather: idx[s*128 + p] = row of (dst=p, slot=s)
                fk = (b * N_TOK + sl.T).reshape(-1).astype(np.int16)
                # v-gather: idx[g*128 + dp*32 + s] = row of (dst=4g+dp, slot=s)
                fv = (b * N_TOK + sl.reshape(G, 4, S)).reshape(-1).astype(np.int16)
                ikm[b, c] = _wrap_idx(fk)
                ivm[b, c] = _wrap_idx(fv)
        in_maps.append(
            {
                "kf": kf,
                "vf": vf,
                "qc": np.ascontiguousarray(qcc),
                "ik": ikm,
                "iv": ivm,
                "oh": ohm,
            }
        )

    res = run_bass_kernel_spmd(nc, in_maps, list(range(N_CORES)))
    out = np.empty((BS, N_TOK, NH, W), dtype=np.float32)
    for core in range(N_CORES):
        lo = core * DT
        r = np.asarray(res.results[core]["oc"]).astype(np.float32)
        # [BS, CH, 128, (w h)] -> [BS, DT, NH, W]
        out[:, lo : lo + DT] = r.reshape(BS, DT, W, NH).transpose(0, 1, 3, 2)
    return out
